# revision 17
# baseline (speedup 1.0000x reference)
"""MCSPN Trainium2 kernel: guidance convs + softmax gates + 4-step CSPN recurrence.

Data-parallel over batch: 8 images -> 8 NeuronCores, one image per core.
Per core:
  phase A: conv3x3 in bf16 (18 accum MMs/row-pair, per-tap column windows so
           feats tiles need no guard columns -> contiguous 8KB DMA packets)
           -> bias+ReLU (ACT, bf16 out) -> conv1x1 bf16 -> exp (ACT, f32r)
           -> per-row DMA scatter into d-major gate layout
           e_all [H=128 part, 4dir, 19k, 256w] (f32r)
  softmax: adds -> reciprocal_approx_fast -> 4 normalize muls, all fp32 on
           DVE (mixed-dtype DVE ops run at half rate, so everything after
           the conv stays 32-bit); boundary gate cols zeroed; up/dn gates
           pre-shifted by one row (PE matmul, written back in place) so the
           recurrence gates BEFORE the shift matmul.
  phase B: h flat-guarded [128, 1+19*256+1] plain f32 (never rounded).
           Per step: a=g0*left (AP-offset view, DVE), b=g1*right (GPSIMD),
           u=g2'*h, v=g3'*h (DVE, f32r out); PE accumulates
           s_up@u + s_dn@v per k into PSUM; fin = (a+b) + psum.
"""
import os
import sys

sys.path.insert(0, "/opt/trn_rl_repo")

import numpy as np

B, CIN, H, W = 8, 256, 128, 256
K = 19
MID = 128
KD = 4 * K  # 76
EPS = 1e-5
T_STEPS = 4
RG = 16          # output rows per feats group
GR = RG + 2      # rows held per group tile (1-row halo each side)
NG = H // RG     # 8
KW = K * W       # 4864
FH = KW + 2      # flat guarded h width
PCH = [(0, 8), (8, 8), (16, 3)]  # k-chunks for PSUM-bound work


def _build():
    import concourse.bacc as bacc
    import concourse.mybir as mybir
    import concourse.tile as tile

    f32 = mybir.dt.float32
    f32r = mybir.dt.float32r
    bf16 = mybir.dt.bfloat16
    Act = mybir.ActivationFunctionType
    Alu = mybir.AluOpType

    nc = bacc.Bacc("TRN2", target_bir_lowering=False)

    feats_d = nc.dram_tensor("feats", [CIN, H, W], bf16, kind="ExternalInput")
    logits_d = nc.dram_tensor("logits", [K, H, W], bf16, kind="ExternalInput")
    w1t_d = nc.dram_tensor("w1t", [128, 2, 9, MID], bf16, kind="ExternalInput")
    bmid_d = nc.dram_tensor("bmid", [MID, 1], f32, kind="ExternalInput")
    w2t_d = nc.dram_tensor("w2t", [MID, KD], bf16, kind="ExternalInput")
    b2_d = nc.dram_tensor("b2", [KD, 1], f32, kind="ExternalInput")
    sup_d = nc.dram_tensor("sup", [128, 128], bf16, kind="ExternalInput")
    sdn_d = nc.dram_tensor("sdn", [128, 128], bf16, kind="ExternalInput")
    out_d = nc.dram_tensor("out", [K, H, W], f32, kind="ExternalOutput")

    # kx -> (ic0, ic1, oc0, oc1): out[:, oc0:oc1] += w[kx].T @ in[:, ic0:ic1]
    WIN = {0: (0, W - 1, 1, W), 1: (0, W, 0, W), 2: (1, W, 0, W - 1)}

    with tile.TileContext(nc) as tc:
        with tc.tile_pool(name="persist", bufs=1) as pp, \
             tc.tile_pool(name="hpool", bufs=1) as hp:
            e_all = pp.tile([128, 4, K, W], bf16)  # d-major gates, 76KB/part
            h_a = hp.tile([128, FH], bf16)
            h_b = hp.tile([128, FH], bf16)
            w1_r = pp.tile([128, 2, 9, MID], bf16)
            w2_r = pp.tile([MID, KD], bf16)
            bmid = pp.tile([MID, 1], f32)
            b2c = pp.tile([KD, 1], f32)
            s_up = pp.tile([128, 128], bf16)
            s_dn = pp.tile([128, 128], bf16)

            # zero guard columns (0 and FH-1) of both h buffers
            nc.vector.memset(h_a[:, 0:FH:FH - 1], 0.0)
            nc.vector.memset(h_b[:, 0:FH:FH - 1], 0.0)
            for (c, t0) in ((0, 3), (1, 3), (0, 0), (1, 0), (0, 6), (1, 6)):
                nc.sync.dma_start(out=w1_r[:, c, t0:t0 + 3],
                                  in_=w1t_d[:, c, t0:t0 + 3])
            # h0 = logits on the scalar queue so phase A's first feats
            # loads aren't stuck behind these issues
            for k in range(K):
                nc.scalar.dma_start(out=h_a[:, 1 + k * W:1 + (k + 1) * W],
                                    in_=logits_d[k])

            # ================= phase A: guidance =================
            with tc.tile_pool(name="frows", bufs=4) as frp, \
                 tc.tile_pool(name="xrow", bufs=4) as xrp, \
                 tc.tile_pool(name="estrip", bufs=8) as esp, \
                 tc.tile_pool(name="psA", bufs=5, space="PSUM") as psA, \
                 tc.tile_pool(name="psG", bufs=3, space="PSUM") as psG:
                ftiles = {}

                def load_group(gi):
                    # tile holds rows gi*RG-1 .. gi*RG+RG (halo both sides);
                    # out-of-range halo rows are zeroed (conv zero-padding)
                    ft = frp.tile([128, 2, GR, W], bf16, name=f"ft{gi}",
                                  tag="ft")
                    r_lo = gi * RG - 1
                    s_lo = 0
                    n = GR
                    if r_lo < 0:
                        nc.vector.memset(ft[:, :, 0:1], 0.0)
                        r_lo, s_lo, n = 0, 1, n - 1
                    if r_lo + n > H:
                        nc.vector.memset(ft[:, :, GR - 1:GR], 0.0)
                        n -= 1
                    for c in range(2):
                        for o in range(0, n, 6):
                            m = min(6, n - o)
                            nc.sync.dma_start(
                                out=ft[:, c, s_lo + o:s_lo + o + m],
                                in_=feats_d[c * 128:(c + 1) * 128,
                                            r_lo + o:r_lo + o + m, :])
                    ftiles[gi] = ft

                load_group(0)
                load_group(1)
                pend = []  # deferred conv1x1 emitter for the previous pair
                nc.sync.dma_start(out=w2_r[:], in_=w2t_d[:])
                nc.sync.dma_start(out=bmid[:], in_=bmid_d[:])
                nc.sync.dma_start(out=b2c[:], in_=b2_d[:])
                nc.sync.dma_start(out=s_up[:], in_=sup_d[:])
                nc.sync.dma_start(out=s_dn[:], in_=sdn_d[:])
                for g in range(NG):
                    if g + 2 < NG:
                        load_group(g + 2)
                    ftg = ftiles[g]
                    for y in range(RG * g, RG * g + RG - 1, 2):
                        acc = psA.tile([MID, 2, W], f32, name="acc")
                        mms = []  # (c, tap, rhs_ap, out_ap)
                        # ky=1 first with kx=1 first: full N=512 start matmul
                        for ky in (1, 0, 2):
                            sl = (y % RG) + ky  # slot of input row y+ky-1
                            kxs = (1, 0, 2) if ky == 1 else (0, 1, 2)
                            for c in range(2):
                                for kx in kxs:
                                    ic0, ic1, oc0, oc1 = WIN[kx]
                                    tap = ky * 3 + kx
                                    mms.append((c, tap,
                                        ftg[:, c, sl:sl + 2, ic0:ic1],
                                        acc[:, 0:2, oc0:oc1]))
                        for i, (c, tap, rhs, oap) in enumerate(mms):
                            nc.tensor.matmul(out=oap,
                                             lhsT=w1_r[:, c, tap, :],
                                             rhs=rhs, start=(i == 0),
                                             stop=(i == len(mms) - 1))
                            if i == 5 and pend:
                                pend.pop()()
                        xr = xrp.tile([MID, 2, W], bf16, name="xr")
                        # relu(x + bias) on the otherwise-idle DVE so the
                        # scalar queue never gates PSUM release
                        nc.vector.tensor_scalar(out=xr[:], in0=acc[:],
                                                scalar1=bmid[:], scalar2=0.0,
                                                op0=Alu.add, op1=Alu.max)

                        def emit_1x1(xr=xr, y=y):
                            accg = psG.tile([KD, 2, W], f32, name="accg")
                            nc.tensor.matmul(out=accg[:], lhsT=w2_r[:],
                                             rhs=xr[:], start=True, stop=True)
                            es = esp.tile([KD, 2, W], bf16, name="es")
                            nc.scalar.activation(es[:], accg[:], Act.Exp,
                                                 bias=b2c[:], scale=1.0)
                            for r in range(2):
                                nc.sync.dma_start(
                                    out=e_all[y + r:y + r + 1, 0:2],
                                    in_=es[0:38, r, :])
                                nc.sync.dma_start(
                                    out=e_all[y + r:y + r + 1, 2:4],
                                    in_=es[38:76, r, :])

                        pend.append(emit_1x1)
                if pend:
                    pend.pop()()

            # ============ softmax + gate pre-shift ============
            with tc.tile_pool(name="smx", bufs=1) as sp, \
                 tc.tile_pool(name="psSM", bufs=2, space="PSUM") as psSM:
                s_all = sp.tile([128, KW], f32)
                t_all = sp.tile([128, KW], f32)
                s3 = s_all[:].rearrange("p (k w) -> p k w", k=K)
                t3 = t_all[:].rearrange("p (k w) -> p k w", k=K)
                ef = [e_all[:, d] for d in range(4)]
                nc.vector.tensor_tensor(out=s3, in0=ef[0], in1=ef[1],
                                        op=Alu.add)
                nc.vector.tensor_tensor(out=t3, in0=ef[2], in1=ef[3],
                                        op=Alu.add)
                nc.vector.tensor_tensor(out=s_all[:], in0=s_all[:],
                                        in1=t_all[:], op=Alu.add)
                # reuse t_all as the reciprocal
                nc.vector.reciprocal_approx_fast(out=t_all[:], in_=s_all[:])
                # normalize up/dn gates first so their pre-shift starts early
                for d in (2, 3, 0, 1):
                    nc.vector.tensor_tensor(out=e_all[:, d], in0=ef[d],
                                            in1=t3, op=Alu.mult)
                # zero boundary gates so flat-h cross-k reads contribute 0
                nc.vector.memset(e_all[:, 0, :, 0:1], 0.0)
                nc.vector.memset(e_all[:, 1, :, W - 1:W], 0.0)
                # g2' = s_dn @ g2 (g2'[p] = g2[p+1]), g3' = s_up @ g3 -- the
                # recurrence gates BEFORE the shift matmul. Written back into
                # e_all[:, 2] / e_all[:, 3] chunk by chunk.
                for (k0, nk) in PCH:
                    for (mat, d) in ((s_dn, 2), (s_up, 3)):
                        psg = psSM.tile([128, 8, W], f32, name="psg")
                        for j in range(0, nk - 1, 2):
                            nc.tensor.matmul(
                                out=psg[:, j:j + 2],
                                lhsT=mat[:],
                                rhs=e_all[:, d, k0 + j:k0 + j + 2],
                                start=True, stop=True)
                        if nk % 2:
                            nc.tensor.matmul(
                                out=psg[:, nk - 1], lhsT=mat[:],
                                rhs=e_all[:, d, k0 + nk - 1],
                                start=True, stop=True)
                        nc.scalar.copy(out=e_all[:, d, k0:k0 + nk],
                                       in_=psg[:, 0:nk])

            # ================= phase B: recurrence =================
            with tc.tile_pool(name="pbt", bufs=1) as tp, \
                 tc.tile_pool(name="psS", bufs=2, space="PSUM") as psS:
                af = tp.tile([128, K, W], bf16)
                bf = tp.tile([128, K, W], bf16)
                uf = tp.tile([128, K, W], bf16)
                vf = tp.tile([128, K, W], bf16)
                o32 = tp.tile([128, K, W], f32)
                g2v = e_all[:, 2]
                g3v = e_all[:, 3]
                g0v = e_all[:, 0]
                g1v = e_all[:, 1]
                cur, nxt = h_a, h_b
                for t in range(T_STEPS):
                    cv = cur[:, 1:1 + KW].rearrange("p (k w) -> p k w", k=K)
                    lv = cur[:, 0:KW].rearrange("p (k w) -> p k w", k=K)
                    rv = cur[:, 2:2 + KW].rearrange("p (k w) -> p k w", k=K)
                    nc.vector.tensor_tensor(out=uf[:], in0=g2v, in1=cv,
                                            op=Alu.mult)
                    nc.vector.tensor_tensor(out=vf[:], in0=g3v, in1=cv,
                                            op=Alu.mult)
                    nc.vector.tensor_tensor(out=af[:], in0=g0v, in1=lv,
                                            op=Alu.mult)
                    nc.vector.tensor_tensor(out=bf[:], in0=g1v, in1=rv,
                                            op=Alu.mult)
                    # ab sum in place of bf
                    nc.vector.tensor_tensor(out=bf[:], in0=af[:], in1=bf[:],
                                            op=Alu.add)
                    last = t == T_STEPS - 1
                    for (k0, nk) in PCH:
                        ps = psS.tile([128, 8, W], f32, name="ps")
                        for j in range(0, nk, 2):
                            nj = min(2, nk - j)
                            kk = k0 + j
                            nc.tensor.matmul(out=ps[:, j:j + nj],
                                             lhsT=s_up[:],
                                             rhs=uf[:, kk:kk + nj],
                                             start=True, stop=False)
                            nc.tensor.matmul(out=ps[:, j:j + nj],
                                             lhsT=s_dn[:],
                                             rhs=vf[:, kk:kk + nj],
                                             start=False, stop=True)
                        if last:
                            oslice = o32[:, k0:k0 + nk]
                        else:
                            oslice = nxt[:, 1 + k0 * W:
                                         1 + (k0 + nk) * W].rearrange(
                                             "p (k w) -> p k w", k=nk)
                        nc.vector.tensor_tensor(out=oslice,
                                                in0=bf[:, k0:k0 + nk],
                                                in1=ps[:, 0:nk], op=Alu.add)
                        if last:
                            for k in range(k0, k0 + nk):
                                nc.sync.dma_start(out=out_d[k],
                                                  in_=o32[:, k])
                    cur, nxt = nxt, cur

    nc.compile()
    return nc


_NC_CACHE = None


def kernel(feats, logits, w1, gamma, beta, mean, var, w2, b2):
    global _NC_CACHE
    from concourse.bass_utils import run_bass_kernel_spmd
    from ml_dtypes import bfloat16

    feats = np.asarray(feats, dtype=np.float32)
    logits = np.asarray(logits, dtype=np.float32)
    w1 = np.asarray(w1, dtype=np.float32)
    w2 = np.asarray(w2, dtype=np.float32)
    b2 = np.asarray(b2, dtype=np.float32)
    gamma = np.asarray(gamma, dtype=np.float32)
    beta = np.asarray(beta, dtype=np.float32)
    mean = np.asarray(mean, dtype=np.float32)
    var = np.asarray(var, dtype=np.float32)

    inv = gamma / np.sqrt(var + EPS)
    w1f = (w1 * inv[:, None, None, None]).astype(np.float32)  # [MID,CIN,3,3]
    bmid = (beta - mean * inv).astype(np.float32)[:, None]    # [MID,1]
    # [cin_in_chunk 128, chunk 2, tap 9, mid 128], bf16
    w1t = (w1f.transpose(1, 2, 3, 0)                  # [CIN,3,3,MID]
           .reshape(2, 128, 9, MID)
           .transpose(1, 0, 2, 3)).astype(bfloat16)
    # d-major channel order: new channel j = d*19+k  <-  old channel k*4+d
    perm = np.array([4 * (j % K) + (j // K) for j in range(KD)])
    w2m = w2.reshape(KD, MID)[perm]
    w2t = np.ascontiguousarray(w2m.T).astype(bfloat16)  # [MID,KD]
    b2c = b2[perm][:, None].astype(np.float32)
    s_up = np.eye(128, k=1, dtype=np.float32).astype(bfloat16)   # out[m]=h[m-1]
    s_dn = np.eye(128, k=-1, dtype=np.float32).astype(bfloat16)  # out[m]=h[m+1]
    feats_bf = feats.astype(bfloat16)
    logits_bf = logits.astype(bfloat16)

    if _NC_CACHE is None:
        _NC_CACHE = _build()
    nc = _NC_CACHE

    in_maps = []
    for i in range(B):
        in_maps.append({
            "feats": np.ascontiguousarray(feats_bf[i]),
            "logits": np.ascontiguousarray(logits_bf[i]),
            "w1t": w1t, "bmid": bmid, "w2t": w2t, "b2": b2c,
            "sup": s_up, "sdn": s_dn,
        })

    trace = bool(os.environ.get("KTRACE"))
    res = run_bass_kernel_spmd(nc, in_maps, list(range(B)), trace=trace)
    if trace and res.exec_time_ns is not None:
        print(f"HW exec time: {res.exec_time_ns} ns")
    out = np.stack([res.results[i]["out"] for i in range(B)], axis=0)
    return out.astype(np.float32)


if __name__ == "__main__":
    rng = np.random.default_rng(0)
    ins = {
        "feats": rng.standard_normal((B, CIN, H, W), dtype=np.float32),
        "logits": rng.standard_normal((B, K, H, W), dtype=np.float32),
        "w1": rng.standard_normal((MID, CIN, 3, 3), dtype=np.float32) / 48.0,
        "gamma": rng.standard_normal(MID).astype(np.float32) * 0.1 + 1.0,
        "beta": rng.standard_normal(MID).astype(np.float32) * 0.1,
        "mean": rng.standard_normal(MID).astype(np.float32) * 0.1,
        "var": rng.random(MID).astype(np.float32) + 0.5,
        "w2": rng.standard_normal((KD, MID, 1, 1)).astype(np.float32) / 11.3,
        "b2": rng.standard_normal(KD).astype(np.float32) * 0.01,
    }
    o = kernel(**ins)
    print("kernel out", o.shape, o.dtype, np.abs(o).mean())


# revision 18
# speedup vs baseline: 1.0128x; 1.0128x over previous
"""MCSPN Trainium2 kernel: guidance convs + softmax gates + 4-step CSPN recurrence.

Data-parallel over batch: 8 images -> 8 NeuronCores, one image per core.
Per core:
  phase A: conv3x3 in bf16 (18 accum MMs/row-pair, per-tap column windows so
           feats tiles need no guard columns -> contiguous 8KB DMA packets)
           -> bias+ReLU (ACT, bf16 out) -> conv1x1 bf16 -> exp (ACT, f32r)
           -> per-row DMA scatter into d-major gate layout
           e_all [H=128 part, 4dir, 19k, 256w] (f32r)
  softmax: adds -> reciprocal_approx_fast -> 4 normalize muls, all fp32 on
           DVE (mixed-dtype DVE ops run at half rate, so everything after
           the conv stays 32-bit); boundary gate cols zeroed; up/dn gates
           pre-shifted by one row (PE matmul, written back in place) so the
           recurrence gates BEFORE the shift matmul.
  phase B: h flat-guarded [128, 1+19*256+1] plain f32 (never rounded).
           Per step: a=g0*left (AP-offset view, DVE), b=g1*right (GPSIMD),
           u=g2'*h, v=g3'*h (DVE, f32r out); PE accumulates
           s_up@u + s_dn@v per k into PSUM; fin = (a+b) + psum.
"""
import os
import sys

sys.path.insert(0, "/opt/trn_rl_repo")

import numpy as np

B, CIN, H, W = 8, 256, 128, 256
K = 19
MID = 128
KD = 4 * K  # 76
EPS = 1e-5
T_STEPS = 4
RG = 16          # output rows per feats group
GR = RG + 2      # rows held per group tile (1-row halo each side)
NG = H // RG     # 8
KW = K * W       # 4864
FH = KW + 2      # flat guarded h width
PCH = [(0, 8), (8, 8), (16, 3)]  # k-chunks for PSUM-bound work


def _build():
    import concourse.bacc as bacc
    import concourse.mybir as mybir
    import concourse.tile as tile

    f32 = mybir.dt.float32
    f32r = mybir.dt.float32r
    bf16 = mybir.dt.bfloat16
    Act = mybir.ActivationFunctionType
    Alu = mybir.AluOpType

    nc = bacc.Bacc("TRN2", target_bir_lowering=False)

    feats_d = nc.dram_tensor("feats", [CIN, H, W], bf16, kind="ExternalInput")
    logits_d = nc.dram_tensor("logits", [K, H, W], bf16, kind="ExternalInput")
    w1t_d = nc.dram_tensor("w1t", [128, 2, 9, MID], bf16, kind="ExternalInput")
    bmid_d = nc.dram_tensor("bmid", [MID, 1], f32, kind="ExternalInput")
    w2t_d = nc.dram_tensor("w2t", [MID, KD], bf16, kind="ExternalInput")
    b2_d = nc.dram_tensor("b2", [KD, 1], f32, kind="ExternalInput")
    sup_d = nc.dram_tensor("sup", [128, 128], bf16, kind="ExternalInput")
    sdn_d = nc.dram_tensor("sdn", [128, 128], bf16, kind="ExternalInput")
    out_d = nc.dram_tensor("out", [K, H, W], f32, kind="ExternalOutput")

    # kx -> (ic0, ic1, oc0, oc1): out[:, oc0:oc1] += w[kx].T @ in[:, ic0:ic1]
    WIN = {0: (0, W - 1, 1, W), 1: (0, W, 0, W), 2: (1, W, 0, W - 1)}

    with tile.TileContext(nc) as tc:
        with tc.tile_pool(name="persist", bufs=1) as pp, \
             tc.tile_pool(name="hpool", bufs=1) as hp:
            e_all = pp.tile([128, 4, K, W], bf16)  # d-major gates, 76KB/part
            h_a = hp.tile([128, FH], bf16)
            h_b = hp.tile([128, FH], bf16)
            w1_r = pp.tile([128, 2, 9, MID], bf16)
            w2_r = pp.tile([MID, KD], bf16)
            bmid = pp.tile([MID, 1], f32)
            b2c = pp.tile([KD, 1], f32)
            s_up = pp.tile([128, 128], bf16)
            s_dn = pp.tile([128, 128], bf16)

            # zero guard columns (0 and FH-1) of both h buffers
            nc.vector.memset(h_a[:, 0:FH:FH - 1], 0.0)
            nc.vector.memset(h_b[:, 0:FH:FH - 1], 0.0)
            nc.sync.dma_start(out=w1_r[:], in_=w1t_d[:])
            # h0 = logits on the scalar queue so phase A's first feats
            # loads aren't stuck behind these issues
            for k in range(K):
                nc.scalar.dma_start(out=h_a[:, 1 + k * W:1 + (k + 1) * W],
                                    in_=logits_d[k])

            # ================= phase A: guidance =================
            with tc.tile_pool(name="frows", bufs=4) as frp, \
                 tc.tile_pool(name="xrow", bufs=4) as xrp, \
                 tc.tile_pool(name="estrip", bufs=8) as esp, \
                 tc.tile_pool(name="psA", bufs=5, space="PSUM") as psA, \
                 tc.tile_pool(name="psG", bufs=3, space="PSUM") as psG:
                ftiles = {}

                def load_group(gi):
                    # tile holds rows gi*RG-1 .. gi*RG+RG (halo both sides);
                    # out-of-range halo rows are zeroed (conv zero-padding)
                    ft = frp.tile([128, 2, GR, W], bf16, name=f"ft{gi}",
                                  tag="ft")
                    r_lo = gi * RG - 1
                    s_lo = 0
                    n = GR
                    if r_lo < 0:
                        nc.vector.memset(ft[:, :, 0:1], 0.0)
                        r_lo, s_lo, n = 0, 1, n - 1
                    if r_lo + n > H:
                        nc.vector.memset(ft[:, :, GR - 1:GR], 0.0)
                        n -= 1
                    for c in range(2):
                        for o in range(0, n, 6):
                            m = min(6, n - o)
                            nc.sync.dma_start(
                                out=ft[:, c, s_lo + o:s_lo + o + m],
                                in_=feats_d[c * 128:(c + 1) * 128,
                                            r_lo + o:r_lo + o + m, :])
                    ftiles[gi] = ft

                load_group(0)
                load_group(1)
                pend = []  # deferred conv1x1 emitter for the previous pair
                nc.sync.dma_start(out=w2_r[:], in_=w2t_d[:])
                nc.sync.dma_start(out=bmid[:], in_=bmid_d[:])
                nc.sync.dma_start(out=b2c[:], in_=b2_d[:])
                nc.sync.dma_start(out=s_up[:], in_=sup_d[:])
                nc.sync.dma_start(out=s_dn[:], in_=sdn_d[:])
                for g in range(NG):
                    if g + 2 < NG:
                        load_group(g + 2)
                    ftg = ftiles[g]
                    for y in range(RG * g, RG * g + RG - 1, 2):
                        acc = psA.tile([MID, 2, W], f32, name="acc")
                        mms = []  # (c, tap, rhs_ap, out_ap)
                        # ky=1 first with kx=1 first: full N=512 start matmul
                        for ky in (1, 0, 2):
                            sl = (y % RG) + ky  # slot of input row y+ky-1
                            kxs = (1, 0, 2) if ky == 1 else (0, 1, 2)
                            for c in range(2):
                                for kx in kxs:
                                    ic0, ic1, oc0, oc1 = WIN[kx]
                                    tap = ky * 3 + kx
                                    mms.append((c, tap,
                                        ftg[:, c, sl:sl + 2, ic0:ic1],
                                        acc[:, 0:2, oc0:oc1]))
                        for i, (c, tap, rhs, oap) in enumerate(mms):
                            nc.tensor.matmul(out=oap,
                                             lhsT=w1_r[:, c, tap, :],
                                             rhs=rhs, start=(i == 0),
                                             stop=(i == len(mms) - 1))
                            if i == 5 and pend:
                                pend.pop()()
                        xr = xrp.tile([MID, 2, W], bf16, name="xr")
                        # relu(x + bias) on the otherwise-idle DVE so the
                        # scalar queue never gates PSUM release
                        nc.vector.tensor_scalar(out=xr[:], in0=acc[:],
                                                scalar1=bmid[:], scalar2=0.0,
                                                op0=Alu.add, op1=Alu.max)

                        def emit_1x1(xr=xr, y=y):
                            accg = psG.tile([KD, 2, W], f32, name="accg")
                            nc.tensor.matmul(out=accg[:], lhsT=w2_r[:],
                                             rhs=xr[:], start=True, stop=True)
                            es = esp.tile([KD, 2, W], bf16, name="es")
                            nc.scalar.activation(es[:], accg[:], Act.Exp,
                                                 bias=b2c[:], scale=1.0)
                            eng = nc.scalar if y >= 104 else nc.sync
                            for r in range(2):
                                eng.dma_start(out=e_all[y + r:y + r + 1],
                                              in_=es[:, r, :])

                        pend.append(emit_1x1)
                if pend:
                    pend.pop()()

            # ============ softmax + gate pre-shift ============
            with tc.tile_pool(name="smx", bufs=1) as sp, \
                 tc.tile_pool(name="psSM", bufs=2, space="PSUM") as psSM:
                s_all = sp.tile([128, KW], f32)
                t_all = sp.tile([128, KW], f32)
                s3 = s_all[:].rearrange("p (k w) -> p k w", k=K)
                t3 = t_all[:].rearrange("p (k w) -> p k w", k=K)
                ef = [e_all[:, d] for d in range(4)]
                nc.vector.tensor_tensor(out=s3, in0=ef[0], in1=ef[1],
                                        op=Alu.add)
                nc.vector.tensor_tensor(out=t3, in0=ef[2], in1=ef[3],
                                        op=Alu.add)
                nc.vector.tensor_tensor(out=s_all[:], in0=s_all[:],
                                        in1=t_all[:], op=Alu.add)
                # reuse t_all as the reciprocal
                nc.vector.reciprocal_approx_fast(out=t_all[:], in_=s_all[:])
                # normalize up/dn gates first so their pre-shift starts early
                for d in (2, 3, 0, 1):
                    nc.vector.tensor_tensor(out=e_all[:, d], in0=ef[d],
                                            in1=t3, op=Alu.mult)
                # zero boundary gates so flat-h cross-k reads contribute 0
                nc.vector.memset(e_all[:, 0, :, 0:1], 0.0)
                nc.vector.memset(e_all[:, 1, :, W - 1:W], 0.0)
                # g2' = s_dn @ g2 (g2'[p] = g2[p+1]), g3' = s_up @ g3 -- the
                # recurrence gates BEFORE the shift matmul. Written back into
                # e_all[:, 2] / e_all[:, 3] chunk by chunk.
                for (k0, nk) in PCH:
                    for (mat, d) in ((s_dn, 2), (s_up, 3)):
                        psg = psSM.tile([128, 8, W], f32, name="psg")
                        for j in range(0, nk - 1, 2):
                            nc.tensor.matmul(
                                out=psg[:, j:j + 2],
                                lhsT=mat[:],
                                rhs=e_all[:, d, k0 + j:k0 + j + 2],
                                start=True, stop=True)
                        if nk % 2:
                            nc.tensor.matmul(
                                out=psg[:, nk - 1], lhsT=mat[:],
                                rhs=e_all[:, d, k0 + nk - 1],
                                start=True, stop=True)
                        nc.scalar.copy(out=e_all[:, d, k0:k0 + nk],
                                       in_=psg[:, 0:nk])

            # ================= phase B: recurrence =================
            with tc.tile_pool(name="pbt", bufs=1) as tp, \
                 tc.tile_pool(name="psS", bufs=2, space="PSUM") as psS:
                af = tp.tile([128, K, W], bf16)
                bf = tp.tile([128, K, W], bf16)
                uf = tp.tile([128, K, W], bf16)
                vf = tp.tile([128, K, W], bf16)
                o32 = tp.tile([128, K, W], f32)
                g2v = e_all[:, 2]
                g3v = e_all[:, 3]
                g0v = e_all[:, 0]
                g1v = e_all[:, 1]
                cur, nxt = h_a, h_b
                for t in range(T_STEPS):
                    cv = cur[:, 1:1 + KW].rearrange("p (k w) -> p k w", k=K)
                    lv = cur[:, 0:KW].rearrange("p (k w) -> p k w", k=K)
                    rv = cur[:, 2:2 + KW].rearrange("p (k w) -> p k w", k=K)
                    nc.vector.tensor_tensor(out=uf[:], in0=g2v, in1=cv,
                                            op=Alu.mult)
                    nc.vector.tensor_tensor(out=vf[:], in0=g3v, in1=cv,
                                            op=Alu.mult)
                    nc.vector.tensor_tensor(out=af[:], in0=g0v, in1=lv,
                                            op=Alu.mult)
                    nc.vector.tensor_tensor(out=bf[:], in0=g1v, in1=rv,
                                            op=Alu.mult)
                    # ab sum in place of bf
                    nc.vector.tensor_tensor(out=bf[:], in0=af[:], in1=bf[:],
                                            op=Alu.add)
                    last = t == T_STEPS - 1
                    for (k0, nk) in PCH:
                        ps = psS.tile([128, 8, W], f32, name="ps")
                        for j in range(0, nk, 2):
                            nj = min(2, nk - j)
                            kk = k0 + j
                            nc.tensor.matmul(out=ps[:, j:j + nj],
                                             lhsT=s_up[:],
                                             rhs=uf[:, kk:kk + nj],
                                             start=True, stop=False)
                            nc.tensor.matmul(out=ps[:, j:j + nj],
                                             lhsT=s_dn[:],
                                             rhs=vf[:, kk:kk + nj],
                                             start=False, stop=True)
                        if last:
                            oslice = o32[:, k0:k0 + nk]
                        else:
                            oslice = nxt[:, 1 + k0 * W:
                                         1 + (k0 + nk) * W].rearrange(
                                             "p (k w) -> p k w", k=nk)
                        nc.vector.tensor_tensor(out=oslice,
                                                in0=bf[:, k0:k0 + nk],
                                                in1=ps[:, 0:nk], op=Alu.add)
                        if last:
                            for k in range(k0, k0 + nk):
                                nc.sync.dma_start(out=out_d[k],
                                                  in_=o32[:, k])
                    cur, nxt = nxt, cur

    nc.compile()
    return nc


_NC_CACHE = None


def kernel(feats, logits, w1, gamma, beta, mean, var, w2, b2):
    global _NC_CACHE
    from concourse.bass_utils import run_bass_kernel_spmd
    from ml_dtypes import bfloat16

    feats = np.asarray(feats, dtype=np.float32)
    logits = np.asarray(logits, dtype=np.float32)
    w1 = np.asarray(w1, dtype=np.float32)
    w2 = np.asarray(w2, dtype=np.float32)
    b2 = np.asarray(b2, dtype=np.float32)
    gamma = np.asarray(gamma, dtype=np.float32)
    beta = np.asarray(beta, dtype=np.float32)
    mean = np.asarray(mean, dtype=np.float32)
    var = np.asarray(var, dtype=np.float32)

    inv = gamma / np.sqrt(var + EPS)
    w1f = (w1 * inv[:, None, None, None]).astype(np.float32)  # [MID,CIN,3,3]
    bmid = (beta - mean * inv).astype(np.float32)[:, None]    # [MID,1]
    # [cin_in_chunk 128, chunk 2, tap 9, mid 128], bf16
    w1t = (w1f.transpose(1, 2, 3, 0)                  # [CIN,3,3,MID]
           .reshape(2, 128, 9, MID)
           .transpose(1, 0, 2, 3)).astype(bfloat16)
    # d-major channel order: new channel j = d*19+k  <-  old channel k*4+d
    perm = np.array([4 * (j % K) + (j // K) for j in range(KD)])
    w2m = w2.reshape(KD, MID)[perm]
    w2t = np.ascontiguousarray(w2m.T).astype(bfloat16)  # [MID,KD]
    b2c = b2[perm][:, None].astype(np.float32)
    s_up = np.eye(128, k=1, dtype=np.float32).astype(bfloat16)   # out[m]=h[m-1]
    s_dn = np.eye(128, k=-1, dtype=np.float32).astype(bfloat16)  # out[m]=h[m+1]
    feats_bf = feats.astype(bfloat16)
    logits_bf = logits.astype(bfloat16)

    if _NC_CACHE is None:
        _NC_CACHE = _build()
    nc = _NC_CACHE

    in_maps = []
    for i in range(B):
        in_maps.append({
            "feats": np.ascontiguousarray(feats_bf[i]),
            "logits": np.ascontiguousarray(logits_bf[i]),
            "w1t": w1t, "bmid": bmid, "w2t": w2t, "b2": b2c,
            "sup": s_up, "sdn": s_dn,
        })

    trace = bool(os.environ.get("KTRACE"))
    res = run_bass_kernel_spmd(nc, in_maps, list(range(B)), trace=trace)
    if trace and res.exec_time_ns is not None:
        print(f"HW exec time: {res.exec_time_ns} ns")
    out = np.stack([res.results[i]["out"] for i in range(B)], axis=0)
    return out.astype(np.float32)


if __name__ == "__main__":
    rng = np.random.default_rng(0)
    ins = {
        "feats": rng.standard_normal((B, CIN, H, W), dtype=np.float32),
        "logits": rng.standard_normal((B, K, H, W), dtype=np.float32),
        "w1": rng.standard_normal((MID, CIN, 3, 3), dtype=np.float32) / 48.0,
        "gamma": rng.standard_normal(MID).astype(np.float32) * 0.1 + 1.0,
        "beta": rng.standard_normal(MID).astype(np.float32) * 0.1,
        "mean": rng.standard_normal(MID).astype(np.float32) * 0.1,
        "var": rng.random(MID).astype(np.float32) + 0.5,
        "w2": rng.standard_normal((KD, MID, 1, 1)).astype(np.float32) / 11.3,
        "b2": rng.standard_normal(KD).astype(np.float32) * 0.01,
    }
    o = kernel(**ins)
    print("kernel out", o.shape, o.dtype, np.abs(o).mean())
